# revision 14
# baseline (speedup 1.0000x reference)
"""Trainium2 Bass kernel for nn_BaseGenerator (4-layer dense transformer).

Strategy: pure data-parallel over batch (B=8 -> 8 NeuronCores, no
collectives).  Each core runs the full transformer on one batch element.
Activations are kept feature-major [E, S] in bf16 so every GEMM contracts
over the partition dim; PSUM accumulates in fp32.

v2 changes vs v1 (all aimed at cutting TensorE cycles / serial stalls):
  - scores: K=64 matmuls for head pairs run concurrently on PE row groups
    (0,0)/(64,0); causal column trim (q >= kc*128) on scores/mask/exp/AV.
  - mask packed to [H, 128, 1280] (valid causal region only).
  - out-proj: two heads' ctx packed into one [128, S] tile -> K=128 mms.
  - V bias folded into bo on host (bo' = bo + Wo @ bv).
  - LN: sums via 2-way col-tiled M=1 matmuls, rstd = exp(-0.5*ln(v))
    (stays in the natural_log_exp activation table set -> fewer table loads),
    dummy K=1 matmul "tickles" keep the PE HAM warm through LN.
  - embedding: one K=94 matmul per chunk (val/ring one-hots stacked).
"""

import os
import sys

for _p in ("/opt/trn_rl_repo",):
    if _p not in sys.path:
        sys.path.insert(0, _p)

import ml_dtypes
import numpy as np

import concourse.bass as bass
import concourse.mybir as mybir
import concourse.tile as tile
from concourse import bacc
from concourse.bass_utils import run_bass_kernel_spmd

BF16 = ml_dtypes.bfloat16

L, E, H, F = 4, 1024, 16, 4096
B, S = 8, 512
VV, VR = 40, 30
DIST_V = 200
PAD_ID = 0
DH = E // H  # 64
NE = E // 128  # 8 feature chunks
NO = 10  # logit row tiles (1280 padded)
NEG = -1.0e9

# causal column layout: for k-chunk kc, valid q range is [kc*128, 512)
KOFF = [0, 512, 896, 1152]
KW = [512, 384, 256, 128]
MASKW = 1280

f32 = mybir.dt.float32
bf16 = mybir.dt.bfloat16
AF = mybir.ActivationFunctionType
OP = mybir.AluOpType

_CACHE = {}


# ----------------------------------------------------------------------------
# host-side input prep
# ----------------------------------------------------------------------------

def _prep_shared(inp):
    """Weight-layout prep shared by all cores. Returns dict name->np array."""
    out = {}

    def b16(x):
        return np.ascontiguousarray(x.astype(BF16))

    Wqkv = np.asarray(inp["Wqkv"], np.float32).copy()  # [L, 3E, E]
    bqkv = np.asarray(inp["bqkv"], np.float32).copy()  # [L, 3E]
    bv = bqkv[:, 2 * E:].copy()  # [L, E] (V bias, folded into bo below)
    # fold attention scale into Q projection
    scale = 1.0 / np.sqrt(DH)
    Wqkv[:, :E, :] *= scale
    bqkv[:, :E] *= scale

    def block_lhsT(W, gsize):
        # W: [L?, OUT, IN] -> [.., G, 128, IN//128, gsize] with
        # out[..., g, p, c, o] = W[..., g*gsize + o, c*128 + p]
        *lead, O, I = W.shape
        G = O // gsize
        nc_ = I // 128
        Wb = W.reshape(*lead, G, gsize, nc_, 128)
        Wb = np.moveaxis(Wb, -1, -3)  # [..., G, 128, gsize, nc]
        Wb = np.swapaxes(Wb, -1, -2)  # [..., G, 128, nc, gsize]
        return np.ascontiguousarray(Wb)

    out["wqkv"] = b16(block_lhsT(Wqkv, 512))          # [L, 6, 128, 8, 512]

    # Wo head-pair major: wo2[l, p, hp, mt, o] = Wo[l, mt*128+o, hp*128+p]
    Wo = np.asarray(inp["Wo"], np.float32)  # [L, E(out), E(in=ctx)]
    t = Wo.reshape(L, 8, 128, 8, 128)       # [l, mt, o, hp, p]
    out["wo2"] = b16(t.transpose(0, 4, 3, 1, 2))  # [l, p, hp, mt, o]

    # bo' = bo + Wo @ bv  (ctx rows are normalized, sum of attn weights = 1)
    bo2 = np.asarray(inp["bo"], np.float32) + np.einsum("loi,li->lo", Wo, bv)

    out["w1"] = b16(block_lhsT(np.asarray(inp["W1"], np.float32), 512))  # [L,8,128,8,512]
    W2 = np.asarray(inp["W2"], np.float32)  # out=E, in=F
    w2b = block_lhsT(W2, 512)  # [L, 2, 128, 32, 512]
    w2b = w2b.reshape(L, 2, 128, 4, 8, 512).transpose(0, 1, 3, 2, 4, 5)
    out["w2"] = b16(w2b)  # [L, 2, 4, 128, 8, 512]

    genW = np.asarray(inp["gen_W"], np.float32)  # [1200, E]
    genW_pad = np.zeros((1280, E), np.float32)
    genW_pad[:1200] = genW
    out["genw"] = b16(block_lhsT(genW_pad, 640))  # [2, 128, 8, 640]

    gen_b = np.asarray(inp["gen_b"], np.float32)
    gbp = np.zeros((1280,), np.float32)
    gbp[:1200] = gen_b
    out["gen_b_pp"] = np.ascontiguousarray(gbp.reshape(NO, 128).T)  # [128, 10]

    def pp(v):  # [..., N*128] -> [..., 128, N]
        *lead, N = v.shape
        return np.ascontiguousarray(
            v.reshape(*lead, N // 128, 128).swapaxes(-1, -2).astype(np.float32)
        )

    out["bqkv_pp"] = pp(bqkv[:, : 2 * E])  # [L, 128, 16] (Q scaled)
    out["bo_pp"] = pp(bo2)  # [L, 128, 8]
    out["b1_pp"] = pp(np.asarray(inp["b1"], np.float32))  # [L, 128, 32]
    out["b2_pp"] = pp(np.asarray(inp["b2"], np.float32))  # [L, 128, 8]

    ln_s = np.stack([np.asarray(inp["ln1_s"], np.float32),
                     np.asarray(inp["ln2_s"], np.float32)], 1)  # [L, 2, E]
    ln_b = np.stack([np.asarray(inp["ln1_b"], np.float32),
                     np.asarray(inp["ln2_b"], np.float32)], 1)
    out["ln_s_pp"] = pp(ln_s)  # [L, 2, 128, 8]
    out["ln_b_pp"] = pp(ln_b)
    out["lnf_s_pp"] = pp(np.asarray(inp["lnf_s"], np.float32))  # [128, 8]
    out["lnf_b_pp"] = pp(np.asarray(inp["lnf_b"], np.float32))

    # stacked embedding table: rows 0:40 val, 40:64 zero, 64:94 ring
    embcat = np.zeros((94, E), np.float32)
    embcat[0:VV] = np.asarray(inp["val_emb"], np.float32)
    embcat[64:64 + VR] = np.asarray(inp["ring_emb"], np.float32)
    out["embcat"] = b16(embcat)

    # iota for the stacked one-hot: 0..39 | -1 x24 | 0..29 | -1 x34
    iota94 = np.full((128, 1), -1.0, np.float32)
    iota94[0:VV, 0] = np.arange(VV)
    iota94[64:64 + VR, 0] = np.arange(VR)
    out["iota94"] = np.ascontiguousarray(iota94)

    out["id128"] = b16(np.eye(128, dtype=np.float32))
    out["ones_col"] = b16(np.ones((128, 1), np.float32))
    return out


def _prep_percore(inp):
    """Per-core tensors: token rows + packed causal attention mask."""
    val = np.asarray(inp["val_sequences"]).astype(np.int64)    # [B, S]
    ring = np.asarray(inp["ring_sequences"]).astype(np.int64)  # [B, S]
    dist = np.asarray(inp["distance_squares"]).astype(np.int64)  # [B, S, S]
    de = np.asarray(inp["dist_emb"], np.float32)  # [200, H]

    # mask[b, h, k, q] = de[dist[b, q, k], h] or NEG
    m = de[dist]                         # [B, S(q), S(k), H]
    m = m.transpose(0, 3, 2, 1)          # [B, H, k, q]
    kk = np.arange(S)
    causal = kk[:, None] <= kk[None, :]  # [k, q] keep where k <= q
    m = np.where(causal[None, None], m, NEG)
    padk = val == PAD_ID  # [B, S]
    m = np.where(padk[:, None, :, None], NEG, m)
    # pack causal region: [B, H, 128, 1280]; chunk kc covers q in [kc*128,512)
    mp = np.empty((B, H, 128, MASKW), np.float32)
    for kc in range(4):
        mp[:, :, :, KOFF[kc]:KOFF[kc] + KW[kc]] = (
            m[:, :, kc * 128:(kc + 1) * 128, kc * 128:]
        )
    mp = np.ascontiguousarray(mp.astype(BF16))

    cores = []
    for b in range(B):
        cores.append({
            "mask": mp[b],
            "valrow": np.ascontiguousarray(val[b].reshape(1, S).astype(BF16)),
            "ringrow": np.ascontiguousarray(ring[b].reshape(1, S).astype(BF16)),
        })
    return cores


# ----------------------------------------------------------------------------
# device program
# ----------------------------------------------------------------------------

def _declare(nc):
    d = {}

    def di(name, shape, dt):
        d[name] = nc.dram_tensor(name, list(shape), dt, kind="ExternalInput").ap()

    di("wqkv", (L, 6, 128, 8, 512), bf16)
    di("wo2", (L, 128, 8, 8, 128), bf16)
    di("w1", (L, 8, 128, 8, 512), bf16)
    di("w2", (L, 2, 4, 128, 8, 512), bf16)
    di("genw", (2, 128, 8, 640), bf16)
    di("gen_b_pp", (128, NO), f32)
    di("bqkv_pp", (L, 128, 16), f32)
    di("bo_pp", (L, 128, 8), f32)
    di("b1_pp", (L, 128, 32), f32)
    di("b2_pp", (L, 128, 8), f32)
    di("ln_s_pp", (L, 2, 128, 8), f32)
    di("ln_b_pp", (L, 2, 128, 8), f32)
    di("lnf_s_pp", (128, 8), f32)
    di("lnf_b_pp", (128, 8), f32)
    di("embcat", (94, E), bf16)
    di("iota94", (128, 1), f32)
    di("id128", (128, 128), bf16)
    di("ones_col", (128, 1), bf16)
    di("mask", (H, 128, MASKW), bf16)
    di("valrow", (1, S), bf16)
    di("ringrow", (1, S), bf16)
    d["logits"] = nc.dram_tensor(
        "logits", [NO, 128, S], f32, kind="ExternalOutput"
    ).ap()
    if os.environ.get("BG_DEBUG"):
        def do(name, shape):
            d[name] = nc.dram_tensor(name, list(shape), bf16,
                                     kind="ExternalOutput").ap()
        do("dbg_h0", (NE, 128, S))
        do("dbg_qk", (16, 128, S))
        do("dbg_v", (4, 128, H, DH + 1))
        do("dbg_at", (2, 128, MASKW))
        do("dbg_cps", (2, DH + 1, S))
        do("dbg_ctx", (8, 128, S))
        do("dbg_r1", (NE, 128, S))
        do("dbg_h1", (NE, 128, S))
        do("dbg_h2", (NE, 128, S))
    return d


def _emit(nc, tc, d, ctx):
    mm = nc.tensor.matmul

    cpool = ctx.enter_context(tc.tile_pool(name="cpool", bufs=1))
    wpool = ctx.enter_context(tc.tile_pool(name="wpool", bufs=3))
    wopool = ctx.enter_context(tc.tile_pool(name="wopool", bufs=1))
    hpool = ctx.enter_context(tc.tile_pool(name="hpool", bufs=17))
    qkpool = ctx.enter_context(tc.tile_pool(name="qkpool", bufs=16))
    vpool = ctx.enter_context(tc.tile_pool(name="vpool", bufs=5))
    maskpool = ctx.enter_context(tc.tile_pool(name="maskpool", bufs=4))
    atpool = ctx.enter_context(tc.tile_pool(name="atpool", bufs=4))
    ctxpool = ctx.enter_context(tc.tile_pool(name="ctxpool", bufs=10))
    ffpool = ctx.enter_context(tc.tile_pool(name="ffpool", bufs=33))
    tmppool = ctx.enter_context(tc.tile_pool(name="tmppool", bufs=4))
    smallf = ctx.enter_context(tc.tile_pool(name="smallf", bufs=8))
    smallb = ctx.enter_context(tc.tile_pool(name="smallb", bufs=4))
    recpool = ctx.enter_context(tc.tile_pool(name="recpool", bufs=3))
    lnbpool = ctx.enter_context(tc.tile_pool(name="lnbpool", bufs=4))
    outpool = ctx.enter_context(tc.tile_pool(name="outpool", bufs=2))
    pppool = ctx.enter_context(tc.tile_pool(name="pppool", bufs=4))

    ps_gemm = ctx.enter_context(tc.tile_pool(name="ps_gemm", bufs=4, space="PSUM"))
    ps_score = ctx.enter_context(tc.tile_pool(name="ps_score", bufs=2, space="PSUM"))
    ps_ctx = ctx.enter_context(tc.tile_pool(name="ps_ctx", bufs=2, space="PSUM"))

    hw = nc.sync  # HWDGE dma engine

    # --- constants -----------------------------------------------------------
    id128 = cpool.tile([128, 128], bf16)
    hw.dma_start(out=id128, in_=d["id128"])
    ones_col = cpool.tile([128, 1], bf16)
    hw.dma_start(out=ones_col, in_=d["ones_col"])
    iota94 = cpool.tile([128, 1], f32)
    hw.dma_start(out=iota94, in_=d["iota94"])
    embcat = cpool.tile([94, E], bf16)
    hw.dma_start(out=embcat, in_=d["embcat"])
    genb_pp = cpool.tile([128, NO], f32)
    hw.dma_start(out=genb_pp, in_=d["gen_b_pp"])
    lnf_s = cpool.tile([128, 8], f32)
    hw.dma_start(out=lnf_s, in_=d["lnf_s_pp"])
    lnf_b = cpool.tile([128, 8], f32)
    hw.dma_start(out=lnf_b, in_=d["lnf_b_pp"])
    eps_t = cpool.tile([128, 1], f32)
    nc.vector.memset(eps_t, 1e-5)

    # --- embedding -----------------------------------------------------------
    with nc.named_scope("embed"):
        vr = tmppool.tile([94, S], bf16, tag="sq")
        nc.vector.memset(vr[32:64, :], -2.0)
        nc.gpsimd.dma_start(out=vr[0:VV, :], in_=d["valrow"].to_broadcast((VV, S)))
        nc.gpsimd.dma_start(out=vr[64:64 + VR, :],
                            in_=d["ringrow"].to_broadcast((VR, S)))
        oh = tmppool.tile([94, S], bf16, tag="tmp")
        nc.vector.tensor_scalar(oh, vr, iota94[0:94, :], None, OP.is_equal)

        h_t = []
        for c in range(NE):
            ps = ps_gemm.tile([128, S], f32, tag="gemm")
            mm(ps, embcat[:, c * 128:(c + 1) * 128], oh, start=True, stop=True)
            ht = hpool.tile([128, S], bf16, tag="h")
            nc.scalar.activation(ht, ps, AF.Copy, scale=float(np.sqrt(E)))
            if "dbg_h0" in d:
                hw.dma_start(out=d["dbg_h0"][c], in_=ht)
            h_t.append(ht)

    # --- layers --------------------------------------------------------------
    env = dict(locals())
    for l in range(L):
        h_t = _layer(nc, tc, d, l, h_t, env)

    # --- final LN + head -----------------------------------------------------
    with nc.named_scope("final"):
        hf = _layernorm(nc, d, h_t, lnf_s, lnf_b, env, "lnf")
        genw_sb = []
        for g in range(2):
            wt = wpool.tile([128, 8, 640], bf16, tag="w")
            hw.dma_start(out=wt, in_=d["genw"][g])
            genw_sb.append(wt)
        for mt in range(NO):
            g, mi = divmod(mt, 5)
            ps = ps_gemm.tile([128, S], f32, tag="gemm")
            for c in range(NE):
                mm(ps, genw_sb[g][:, c, mi * 128:(mi + 1) * 128], hf[c],
                   start=(c == 0), stop=(c == NE - 1))
            ot = outpool.tile([128, S], f32, tag="f32out")
            nc.scalar.activation(ot, ps, AF.Identity, bias=genb_pp[:, mt:mt + 1])
            hw.dma_start(out=d["logits"][mt], in_=ot)


def _layernorm(nc, d, r_t, s_pp, b_pp, env, nm):
    """r_t: 8 bf16 [128, S] feature-major tiles -> returns normalized tiles.

    sums via 2-way col-tiled M=1 matmuls; rstd = exp(-0.5 * ln(var + eps))
    to stay inside the natural_log_exp activation table set.
    """
    mm = nc.tensor.matmul
    ps_ctx = env["ps_ctx"]; ps_score = env["ps_score"]
    smallf = env["smallf"]; smallb = env["smallb"]
    tmppool = env["tmppool"]; hpool = env["hpool"]; lnbpool = env["lnbpool"]
    ones_col = env["ones_col"]

    psS = ps_ctx.tile([65, S], f32, tag="ctxps", name=f"{nm}_psS")
    psQ = ps_ctx.tile([65, S], f32, tag="ctxps", name=f"{nm}_psQ")
    for p in range(4):
        c0, c1 = 2 * p, 2 * p + 1
        sq0 = tmppool.tile([128, S], bf16, tag="sq")
        nc.vector.tensor_mul(sq0, r_t[c0], r_t[c0])
        sq1 = tmppool.tile([128, S], bf16, tag="tmp")
        nc.vector.tensor_mul(sq1, r_t[c1], r_t[c1])
        st, sp = (p == 0), (p == 3)
        mm(psS[0:1, :], ones_col, r_t[c0], start=st, stop=sp)
        mm(psS[64:65, :], ones_col, r_t[c1], start=st, stop=sp)
        mm(psQ[0:1, :], ones_col, sq0, start=st, stop=sp)
        mm(psQ[64:65, :], ones_col, sq1, start=st, stop=sp)

    # combine the two col-tiled partials (only one PSUM operand per DVE op)
    sh = smallf.tile([1, S], f32, tag="sf", name=f"{nm}_sh")
    nc.scalar.activation(sh, psS[0:1, :], AF.Copy)
    sums = smallf.tile([1, S], f32, tag="sf", name=f"{nm}_sums")
    nc.vector.scalar_tensor_tensor(sums, psS[64:65, :], 1.0, sh,
                                   OP.mult, OP.add)
    qh = smallf.tile([1, S], f32, tag="sf", name=f"{nm}_qh")
    nc.scalar.activation(qh, psQ[0:1, :], AF.Copy)
    sumq = smallf.tile([1, S], f32, tag="sf", name=f"{nm}_sumq")
    nc.vector.scalar_tensor_tensor(sumq, psQ[64:65, :], 1.0, qh,
                                   OP.mult, OP.add)

    s2 = smallf.tile([1, S], f32, tag="sf", name=f"{nm}_s2")
    nc.scalar.activation(s2, sums, AF.Square)
    varE = smallf.tile([1, S], f32, tag="sf", name=f"{nm}_varE")
    # varE = sumsq - s2/E  (= E * var)
    nc.vector.scalar_tensor_tensor(varE, s2, -1.0 / E, sumq, OP.mult, OP.add)
    lnv = smallf.tile([1, S], f32, tag="sf", name=f"{nm}_lnv")
    nc.scalar.activation(lnv, varE, AF.Ln, scale=1.0 / E,
                         bias=env["eps_t"][0:1, :])
    rstd = smallf.tile([1, S], f32, tag="sf", name=f"{nm}_rstd")
    nc.scalar.activation(rstd, lnv, AF.Exp, scale=-0.5)

    rstd_b = smallb.tile([1, S], bf16, tag="sb")
    nc.vector.tensor_copy(rstd_b, rstd)
    u_b = smallb.tile([1, S], bf16, tag="sb")
    # u = mean * rstd = (sum/E) * rstd
    nc.vector.scalar_tensor_tensor(u_b, sums, 1.0 / E, rstd, OP.mult, OP.mult)

    # PE warmth tickles (data-dependent on mid-LN tiles, so they execute
    # spread through the LN window and keep the HAM from re-throttling)
    tick = ps_score.tile([1, 128], f32, tag="score", name=f"{nm}_t0")
    mm(tick, ones_col[0:1, :], rstd_b[:, 0:128], start=True, stop=True)

    rstdR = lnbpool.tile([128, S], bf16, tag="lnb", name=f"{nm}_rstdR")
    nc.gpsimd.partition_broadcast(rstdR, rstd_b, channels=128)
    uR = lnbpool.tile([128, S], bf16, tag="lnb", name=f"{nm}_uR")
    nc.gpsimd.partition_broadcast(uR, u_b, channels=128)

    tick2 = ps_score.tile([1, 128], f32, tag="score", name=f"{nm}_t1")
    mm(tick2, ones_col, uR[:, 0:128], start=True, stop=True)

    out_t = []
    for c in range(NE):
        t1 = tmppool.tile([128, S], bf16, tag="tmp")
        nc.vector.tensor_mul(t1, r_t[c], rstdR)
        t2 = tmppool.tile([128, S], bf16, tag="sq")
        nc.vector.tensor_sub(t2, t1, uR)
        ht = hpool.tile([128, S], bf16, tag="h")
        nc.scalar.activation(ht, t2, AF.Identity,
                             bias=b_pp[:, c:c + 1], scale=s_pp[:, c:c + 1])
        out_t.append(ht)
    return out_t


def _layer(nc, tc, d, l, h_t, env):
    mm = nc.tensor.matmul
    hw = nc.sync
    wpool = env["wpool"]; wopool = env["wopool"]; hpool = env["hpool"]
    qkpool = env["qkpool"]; vpool = env["vpool"]; maskpool = env["maskpool"]
    atpool = env["atpool"]; ctxpool = env["ctxpool"]; ffpool = env["ffpool"]
    tmppool = env["tmppool"]; smallf = env["smallf"]; recpool = env["recpool"]
    pppool = env["pppool"]
    ps_gemm = env["ps_gemm"]; ps_score = env["ps_score"]; ps_ctx = env["ps_ctx"]
    id128 = env["id128"]

    # per-layer small params
    bqkv_pp = pppool.tile([128, 16], f32, tag="pp16")
    hw.dma_start(out=bqkv_pp, in_=d["bqkv_pp"][l])
    bo_pp = pppool.tile([128, 8], f32, tag="pp8")
    hw.dma_start(out=bo_pp, in_=d["bo_pp"][l])
    b1_pp = pppool.tile([128, 32], f32, tag="pp32")
    hw.dma_start(out=b1_pp, in_=d["b1_pp"][l])
    b2_pp = pppool.tile([128, 8], f32, tag="pp8")
    hw.dma_start(out=b2_pp, in_=d["b2_pp"][l])
    ln_s = [pppool.tile([128, 8], f32, tag="pp8", name=f"lns{l}_{i}")
            for i in range(2)]
    ln_b = [pppool.tile([128, 8], f32, tag="pp8", name=f"lnb{l}_{i}")
            for i in range(2)]
    for i in range(2):
        hw.dma_start(out=ln_s[i], in_=d["ln_s_pp"][l, i])
        hw.dma_start(out=ln_b[i], in_=d["ln_b_pp"][l, i])
    # out-proj weights (used mid-layer; DMA overlaps the QKV phase)
    wo_sb = wopool.tile([128, 8, 8, 128], bf16, tag="wo")
    hw.dma_start(out=wo_sb, in_=d["wo2"][l])

    # --- QKV -----------------------------------------------------------------
    with nc.named_scope(f"L{l}_qkv"):
        qk_t = []  # 16 tiles: q 0..7, k 8..15
        for g in range(4):  # Q, K feature-major
            wt = wpool.tile([128, 8, 512], bf16, tag="w")
            hw.dma_start(out=wt, in_=d["wqkv"][l, g])
            for mi in range(4):
                mt = g * 4 + mi
                ps = ps_gemm.tile([128, S], f32, tag="gemm")
                for c in range(NE):
                    mm(ps, wt[:, c, mi * 128:(mi + 1) * 128], h_t[c],
                       start=(c == 0), stop=(c == NE - 1))
                qk = qkpool.tile([128, S], bf16, tag="qk")
                nc.scalar.activation(qk, ps, AF.Identity,
                                     bias=bqkv_pp[:, mt:mt + 1])
                if l == 0 and "dbg_qk" in d:
                    hw.dma_start(out=d["dbg_qk"][mt], in_=qk)
                qk_t.append(qk)
        # V token-major [128, H, DH+1], ones in last column (softmax denom)
        v_t = []
        for n in range(4):
            vt = vpool.tile([128, H, DH + 1], bf16, tag="v")
            nc.vector.memset(vt[:, :, DH:DH + 1], 1.0)
            v_t.append(vt)
        for g in range(2):
            wt = wpool.tile([128, 8, 512], bf16, tag="w")
            hw.dma_start(out=wt, in_=d["wqkv"][l, 4 + g])
            for n in range(4):
                ps = ps_gemm.tile([128, S], f32, tag="gemm")
                for c in range(NE):
                    mm(ps, h_t[c][:, n * 128:(n + 1) * 128], wt[:, c, :],
                       start=(c == 0), stop=(c == NE - 1))
                nc.scalar.activation(
                    v_t[n][:, g * 8:(g + 1) * 8, 0:DH],
                    ps.rearrange("p (a b) -> p a b", a=8), AF.Copy)

    if l == 0 and "dbg_v" in d:
        for n in range(4):
            hw.dma_start(out=d["dbg_v"][n], in_=v_t[n])

    # --- attention ------------------------------------------------------------
    with nc.named_scope(f"L{l}_attn"):
        ctx_pairs = [ctxpool.tile([128, S], bf16, tag="ctx", name=f"cp{l}_{j}")
                     for j in range(8)]
        at_tiles = {}
        mask_sb = {}

        def load_masks(j):
            for h in (2 * j, 2 * j + 1):
                mk = maskpool.tile([128, MASKW], bf16, tag="mask",
                                   name=f"mk{l}_{h}")
                hw.dma_start(out=mk, in_=d["mask"][h])
                mask_sb[h] = mk

        def emit_scores(j):
            qt = qk_t[j]
            kt = qk_t[8 + j]
            for h in (2 * j, 2 * j + 1):
                at_tiles[h] = atpool.tile([128, MASKW], bf16, tag="at",
                                          name=f"at{l}_{h}")
            sps = {}
            for kc in range(4):
                w = KW[kc]
                q0 = kc * 128
                # the two heads' K=64 matmuls run on PE row groups 0 / 64
                for i, h in enumerate((2 * j, 2 * j + 1)):
                    r0 = 64 * i
                    ps = ps_score.tile([128, S], f32, tag="score",
                                       name=f"s{l}_{h}_{kc}")
                    mm(ps[:, 0:w], kt[r0:r0 + DH, q0:q0 + 128],
                       qt[r0:r0 + DH, q0:], start=True, stop=False)
                    sps[(h, kc)] = ps
                for h in (2 * j, 2 * j + 1):
                    ps = sps[(h, kc)]
                    mm(ps[:, 0:w], id128,
                       mask_sb[h][:, KOFF[kc]:KOFF[kc] + w],
                       start=False, stop=True)
                    nc.scalar.activation(
                        at_tiles[h][:, KOFF[kc]:KOFF[kc] + w],
                        ps[:, 0:w], AF.Exp)
            mask_sb.pop(2 * j), mask_sb.pop(2 * j + 1)

        def emit_av(h):
            at = at_tiles.pop(h)
            if l == 0 and h < 2 and "dbg_at" in d:
                hw.dma_start(out=d["dbg_at"][h], in_=at)
            cps = ps_ctx.tile([DH + 1, S], f32, tag="ctxps", name=f"c{l}_{h}")
            for kc in range(4):
                w = KW[kc]
                mm(cps[:, kc * 128:], v_t[kc][:, h, :],
                   at[:, KOFF[kc]:KOFF[kc] + w],
                   start=(kc == 0), stop=(kc == 3))
            if l == 0 and h < 2 and "dbg_cps" in d:
                csb = tmppool.tile([DH + 1, S], bf16, tag="tmp",
                                   name=f"dbgc{h}")
                nc.scalar.activation(csb, cps, AF.Copy)
                hw.dma_start(out=d["dbg_cps"][h], in_=csb)
            srow = smallf.tile([1, S], f32, tag="sf", name=f"sr{l}_{h}")
            nc.vector.tensor_copy(srow, cps[DH:DH + 1, :])
            rec = smallf.tile([1, S], f32, tag="sf", name=f"re{l}_{h}")
            nc.vector.reciprocal_approx_fast(out=rec, in_=srow)
            recR = recpool.tile([DH, S], f32, tag="rec", name=f"rr{l}_{h}")
            nc.gpsimd.partition_broadcast(recR, rec, channels=DH)
            r0 = (h % 2) * 64
            nc.vector.tensor_mul(ctx_pairs[h // 2][r0:r0 + 64, :],
                                 cps[0:DH, :], recR)

        # out-proj group A (mt 0..3), interleaved per ctx pair
        pssA = [ps_gemm.tile([128, S], f32, tag="gemm", name=f"opA{l}_{i}")
                for i in range(4)]

        def emit_oproj_A(hp):
            for mi in range(4):
                mm(pssA[mi], wo_sb[:, hp, mi, :], ctx_pairs[hp],
                   start=(hp == 0), stop=(hp == 7))

        load_masks(0)
        for j in range(8):
            if j < 7:
                load_masks(j + 1)
            emit_scores(j)
            if j >= 1:
                emit_av(2 * (j - 1))
                emit_av(2 * (j - 1) + 1)
                emit_oproj_A(j - 1)
        emit_av(14)
        emit_av(15)
        emit_oproj_A(7)

        r1_t = [None] * 8
        def evict_oproj(pss, grp):
            for mi in range(4):
                mt = grp * 4 + mi
                rtmp = tmppool.tile([128, S], bf16, tag="tmp")
                nc.scalar.activation(rtmp, pss[mi], AF.Identity,
                                     bias=bo_pp[:, mt:mt + 1])
                r1 = hpool.tile([128, S], bf16, tag="h")
                nc.vector.tensor_add(r1, rtmp, h_t[mt])
                if l == 0 and "dbg_r1" in d:
                    hw.dma_start(out=d["dbg_r1"][mt], in_=r1)
                r1_t[mt] = r1

        evict_oproj(pssA, 0)
        pssB = [ps_gemm.tile([128, S], f32, tag="gemm", name=f"opB{l}_{i}")
                for i in range(4)]
        for hp in range(8):
            for mi in range(4):
                mm(pssB[mi], wo_sb[:, hp, 4 + mi, :], ctx_pairs[hp],
                   start=(hp == 0), stop=(hp == 7))
        evict_oproj(pssB, 1)

        if l == 0 and "dbg_ctx" in d:
            for j in range(8):
                hw.dma_start(out=d["dbg_ctx"][j], in_=ctx_pairs[j])

    with nc.named_scope(f"L{l}_ln1"):
        h1_t = _layernorm(nc, d, r1_t, ln_s[0], ln_b[0], env, f"ln{l}_1")
        if l == 0 and "dbg_h1" in d:
            for c in range(NE):
                hw.dma_start(out=d["dbg_h1"][c], in_=h1_t[c])

    # --- FFN -----------------------------------------------------------------
    with nc.named_scope(f"L{l}_ffn"):
        ff_t = []
        for g in range(8):
            wt = wpool.tile([128, 8, 512], bf16, tag="w")
            hw.dma_start(out=wt, in_=d["w1"][l, g])
            for mi in range(4):
                mt = g * 4 + mi
                ps = ps_gemm.tile([128, S], f32, tag="gemm")
                for c in range(NE):
                    mm(ps, wt[:, c, mi * 128:(mi + 1) * 128], h1_t[c],
                       start=(c == 0), stop=(c == NE - 1))
                ft = ffpool.tile([128, S], bf16, tag="ff")
                nc.scalar.activation(ft, ps, AF.Gelu,
                                     bias=b1_pp[:, mt:mt + 1])
                ff_t.append(ft)
        r2_t = [None] * NE
        for half in range(2):
            pss = [ps_gemm.tile([128, S], f32, tag="gemm",
                                name=f"ff2ps{l}_{half}_{i}") for i in range(4)]
            for cg in range(4):
                wt = wpool.tile([128, 8, 512], bf16, tag="w")
                hw.dma_start(out=wt, in_=d["w2"][l, half, cg])
                for c8 in range(8):
                    c = cg * 8 + c8
                    for mi in range(4):
                        mm(pss[mi], wt[:, c8, mi * 128:(mi + 1) * 128], ff_t[c],
                           start=(c == 0), stop=(c == 31))
            for mi in range(4):
                mt = half * 4 + mi
                rtmp = tmppool.tile([128, S], bf16, tag="tmp")
                nc.scalar.activation(rtmp, pss[mi], AF.Identity,
                                     bias=b2_pp[:, mt:mt + 1])
                r2 = hpool.tile([128, S], bf16, tag="h")
                nc.vector.tensor_add(r2, rtmp, h1_t[mt])
                r2_t[mt] = r2

    with nc.named_scope(f"L{l}_ln2"):
        h2_t = _layernorm(nc, d, r2_t, ln_s[1], ln_b[1], env, f"ln{l}_2")
        if l == 0 and "dbg_h2" in d:
            for c in range(NE):
                hw.dma_start(out=d["dbg_h2"][c], in_=h2_t[c])
    return h2_t


def _build():
    if "nc" in _CACHE:
        return _CACHE["nc"]
    from contextlib import ExitStack

    nc = bacc.Bacc("TRN2", debug=False)
    d = _declare(nc)
    with tile.TileContext(nc) as tc:
        with ExitStack() as ctx:
            _emit(nc, tc, d, ctx)
    nc.compile()
    _CACHE["nc"] = nc
    return nc


def kernel_internal(inputs, trace=False, trace_kwargs=None):
    shared = _prep_shared(inputs)
    cores = _prep_percore(inputs)
    nc = _build()
    in_maps = []
    for b in range(B):
        m = dict(shared)
        m.update(cores[b])
        in_maps.append(m)
    res = run_bass_kernel_spmd(
        nc, in_maps, core_ids=list(range(B)), trace=trace,
        **(trace_kwargs or {}),
    )
    outs = []
    for b in range(B):
        lo = res.results[b]["logits"]  # [10, 128, 512]
        lo = lo.reshape(NO * 128, S)[:VV * VR].T  # [512, 1200]
        outs.append(lo)
    out = np.stack(outs).astype(np.float32)  # [B, S, 1200]
    return out, res


def kernel(**inputs):
    out, _ = kernel_internal(inputs)
    return out
